# revision 1
# baseline (speedup 1.0000x reference)
"""Deformable conv (DCNv2) + BN + ReLU Trainium2 Bass kernel.

Sharding: 8 cores = (2 batches) x (4 H-strips of 32 rows). Each core:
  1. 3x3 offset/mask conv via PE matmuls (pixel-major output, bias via
     ones-channel trick).
  2. Bilinear sample positions -> per-pixel patch indices + 4 cell weights
     (DVE, batched over the whole strip).
  3. Gather 2x2x64ch patches from a precomputed patch buffer in DRAM via
     indirect DMA (1KB/descriptor).
  4. Weight cells (DVE), fold, PE-transpose to channel-major, main conv via
     PE matmuls accumulating in PSUM.
  5. BN stats partial sums -> AllReduce across 8 cores -> scale/shift + ReLU.

Host side prepares per-core staged inputs (slab with halo+padding+ones row,
patch buffer, constants) and reassembles the output.
"""

import os
import numpy as np
import concourse.bass as bass
import concourse.mybir as mybir
import concourse.tile as tile
from concourse.bass_utils import run_bass_kernel_spmd
from contextlib import ExitStack

F32 = mybir.dt.float32
I32 = mybir.dt.int32

B, C, O, H, W = 2, 64, 64, 128, 128
NCORES = 8
RPC = H // 4            # rows per core (4 strips per batch)
NPIX_TOT = B * H * W    # 32768 (BN denominator)
BN_EPS = 1e-5


def _sap(ap, off_elems, dims):
    """AP with same tensor/partition dim, custom free dims."""
    return bass.AP(ap.tensor, ap.offset + off_elems, [ap.ap[0]] + dims)


def fix_multiwait(nc):
    """This env's walrus allows only ONE sem wait per instruction; split
    extras into single-wait drains on the same engine immediately before."""
    for f in nc.m.functions:
        for blk in f.blocks:
            i = 0
            while i < len(blk.instructions):
                ins = blk.instructions[i]
                si = ins.sync_info
                if si is not None and si.on_wait and len(si.on_wait) > 1:
                    waits = list(si.on_wait)
                    si.on_wait = [waits[-1]]
                    for j, w in enumerate(waits[:-1]):
                        d2 = mybir.InstDrain(
                            name=f"{ins.name}-wsplit{j}", ins=[], outs=[],
                            engine=ins.engine,
                        )
                        d2.sync_info = mybir.SyncInfo(on_wait=[w], on_update=[])
                        blk.instructions.insert(i, d2)
                        i += 1
                i += 1


def build_nc(T=RPC, collective=True, fixup=True):
    """Build the per-core Bass module. T = number of row-tiles (32 normally)."""
    nc = bass.Bass()
    K9 = 9
    NI = K9 * 128          # gather indices per row-tile
    ELEM = 256             # 2x2 patch x 64 ch, f32

    # ---- per-core external inputs (host-staged) ----
    xslab = nc.dram_tensor("xslab", [C + 1, T + 2, W + 2], F32, kind="ExternalInput")
    pbc = nc.dram_tensor("pbc", [4 * 4096, ELEM], F32, kind="ExternalInput")
    cy = nc.dram_tensor("cy", [128, T, K9], F32, kind="ExternalInput")
    cx = nc.dram_tensor("cx", [128, T, K9], F32, kind="ExternalInput")
    wcat = nc.dram_tensor("wcat", [C + 1, K9, 27], F32, kind="ExternalInput")
    w2 = nc.dram_tensor("w2", [C, K9, O], F32, kind="ExternalInput")
    ident = nc.dram_tensor("ident", [128, 128], F32, kind="ExternalInput")
    gb = nc.dram_tensor("gb", [O, 2], F32, kind="ExternalInput")
    outd = nc.dram_tensor("outn", [O, T, W], F32, kind="ExternalOutput")

    with tile.TileContext(nc) as tc:
        with ExitStack() as ctx:
            cpool = ctx.enter_context(tc.tile_pool(name="const", bufs=1))
            ppool = ctx.enter_context(tc.tile_pool(name="persist", bufs=1))
            wpool = ctx.enter_context(tc.tile_pool(name="wtmp", bufs=1))
            gpool = ctx.enter_context(tc.tile_pool(name="gath", bufs=3))
            mpool = ctx.enter_context(tc.tile_pool(name="mac", bufs=2))
            psA = ctx.enter_context(tc.tile_pool(name="psA", bufs=2, space="PSUM"))
            psB = ctx.enter_context(tc.tile_pool(name="psB", bufs=2, space="PSUM"))
            dpool = ctx.enter_context(tc.tile_pool(name="dram", bufs=1, space="DRAM"))

            TT = nc.vector.tensor_tensor
            TS = nc.vector.tensor_scalar
            STT = nc.vector.scalar_tensor_tensor
            AL = mybir.AluOpType

            # ---- prologue loads ----
            xs = cpool.tile([C + 1, T + 2, W + 2], F32, tag="xs")
            nc.sync.dma_start(xs[:], xslab[:])
            wc = cpool.tile([C + 1, K9, 27], F32, tag="wc")
            nc.sync.dma_start(wc[:], wcat[:])
            w2s = cpool.tile([C, K9, O], F32, tag="w2s")
            nc.sync.dma_start(w2s[:], w2[:])
            idt = cpool.tile([128, 128], F32, tag="idt")
            nc.sync.dma_start(idt[:], ident[:])
            cys = cpool.tile([128, T, K9], F32, tag="cys")
            nc.sync.dma_start(cys[:], cy[:])
            cxs = cpool.tile([128, T, K9], F32, tag="cxs")
            nc.sync.dma_start(cxs[:], cx[:])
            gbs = cpool.tile([O, 2], F32, tag="gbs")
            nc.sync.dma_start(gbs[:], gb[:])
            epst = cpool.tile([128, 1], F32, tag="epst")
            nc.vector.memset(epst[:], BN_EPS)

            # ---- persistent tiles ----
            OFF = ppool.tile([128, T, 27], F32, tag="OFF")
            MK = ppool.tile([128, T, K9], F32, tag="MK")
            W4 = ppool.tile([128, T, K9, 4], F32, tag="W4")
            IDX = ppool.tile([128, T, K9], I32, tag="IDX")
            ST1 = ppool.tile([O, T], F32, tag="ST1")
            ST2 = ppool.tile([O, T], F32, tag="ST2")
            OPRE = ppool.tile([O, T, W], F32, tag="OPRE")
            ON = ppool.tile([O, T, W], F32, tag="ON")

            # ---- phase 1: offset/mask conv (pixel-major out) ----
            for t in range(T):
                pso = psA.tile([128, 27], F32, tag="big")
                for k in range(K9):
                    ky, kx = k // 3, k % 3
                    lhsT = _sap(xs[:], (t + ky) * (W + 2) + kx, [[1, 128]])
                    rhs = _sap(wc[:], k * 27, [[1, 27]])
                    nc.tensor.matmul(pso[:], lhsT, rhs,
                                     start=(k == 0), stop=(k == K9 - 1))
                nc.vector.tensor_copy(OFF[:, t, :], pso[:])
                nc.scalar.activation(MK[:, t, :], OFF[:, t, 18:27],
                                     mybir.ActivationFunctionType.Sigmoid)

            # ---- phase 2: sample coords -> weights + indices (batched) ----
            NF = T * K9
            d3 = lambda: wpool.tile([128, T, K9], F32)

            def floorfix(dst_fl, src, tag):
                """dst_fl = floor(src) via round-to-i32 + fix."""
                ri = wpool.tile([128, T, K9], I32, tag=f"ri_{tag}")
                nc.vector.tensor_copy(ri[:], src)
                rf = wpool.tile([128, T, K9], F32, tag=f"rf_{tag}")
                nc.vector.tensor_copy(rf[:], ri[:])
                g = wpool.tile([128, T, K9], F32, tag=f"g_{tag}")
                TT(g[:], rf[:], src, AL.is_gt)
                TT(dst_fl, rf[:], g[:], AL.subtract)

            offy = _sap(OFF[:], 0, [[27, T], [2, K9]])
            offx = _sap(OFF[:], 1, [[27, T], [2, K9]])
            py = wpool.tile([128, T, K9], F32, tag="py")
            TT(py[:], offy, cys[:], AL.add)
            px = wpool.tile([128, T, K9], F32, tag="px")
            TT(px[:], offx, cxs[:], AL.add)

            Y0 = wpool.tile([128, T, K9], F32, tag="Y0")
            floorfix(Y0[:], py[:], "y")
            X0 = wpool.tile([128, T, K9], F32, tag="X0")
            floorfix(X0[:], px[:], "x")
            WY = wpool.tile([128, T, K9], F32, tag="WY")
            TT(WY[:], py[:], Y0[:], AL.subtract)
            WX = wpool.tile([128, T, K9], F32, tag="WX")
            TT(WX[:], px[:], X0[:], AL.subtract)

            def vpair(F0, wfrac, tag):
                """V0/V1 weights for a dim with floor F0, frac wfrac."""
                a1 = wpool.tile([128, T, K9], F32, tag=f"a1_{tag}")
                TS(a1[:], F0, 0.0, None, AL.is_ge)
                a2 = wpool.tile([128, T, K9], F32, tag=f"a2_{tag}")
                TS(a2[:], F0, 126.0, None, AL.is_le)
                A = wpool.tile([128, T, K9], F32, tag=f"A_{tag}")
                TT(A[:], a1[:], a2[:], AL.mult)
                Bq = wpool.tile([128, T, K9], F32, tag=f"B_{tag}")
                TS(Bq[:], F0, -1.0, None, AL.is_equal)
                Cq = wpool.tile([128, T, K9], F32, tag=f"C_{tag}")
                TS(Cq[:], F0, 127.0, None, AL.is_equal)
                om = wpool.tile([128, T, K9], F32, tag=f"om_{tag}")
                TS(om[:], wfrac, -1.0, 1.0, AL.mult, AL.add)
                u1 = wpool.tile([128, T, K9], F32, tag=f"u1_{tag}")
                TT(u1[:], om[:], A[:], AL.mult)
                u2 = wpool.tile([128, T, K9], F32, tag=f"u2_{tag}")
                TT(u2[:], wfrac, Bq[:], AL.mult)
                V0 = wpool.tile([128, T, K9], F32, tag=f"V0_{tag}")
                TT(V0[:], u1[:], u2[:], AL.add)
                u3 = wpool.tile([128, T, K9], F32, tag=f"u3_{tag}")
                TT(u3[:], wfrac, A[:], AL.mult)
                u4 = wpool.tile([128, T, K9], F32, tag=f"u4_{tag}")
                TT(u4[:], om[:], Cq[:], AL.mult)
                V1 = wpool.tile([128, T, K9], F32, tag=f"V1_{tag}")
                TT(V1[:], u3[:], u4[:], AL.add)
                return V0, V1

            V0, V1 = vpair(Y0[:], WY[:], "vy")
            U0, U1 = vpair(X0[:], WX[:], "ux")
            # fold mask into V
            TT(V0[:], V0[:], MK[:], AL.mult)
            TT(V1[:], V1[:], MK[:], AL.mult)

            def clampfloor2(F0, tag):
                """returns (FB = clamp(F0,0,126), I = floor(FB/2))"""
                c1 = wpool.tile([128, T, K9], F32, tag=f"c1_{tag}")
                TS(c1[:], F0, 0.0, None, AL.max)
                FB = wpool.tile([128, T, K9], F32, tag=f"FB_{tag}")
                TS(FB[:], c1[:], 126.0, None, AL.min)
                h = wpool.tile([128, T, K9], F32, tag=f"h_{tag}")
                TS(h[:], FB[:], 0.5, None, AL.mult)
                Iq = wpool.tile([128, T, K9], F32, tag=f"I_{tag}")
                floorfix(Iq[:], h[:], f"cf_{tag}")
                return FB, Iq

            YB, IY = clampfloor2(Y0[:], "yb")
            XB, IX = clampfloor2(X0[:], "xb")
            PP = wpool.tile([128, T, K9], F32, tag="PP")
            STT(PP[:], IY[:], -2.0, YB[:], AL.mult, AL.add)
            QQ = wpool.tile([128, T, K9], F32, tag="QQ")
            STT(QQ[:], IX[:], -2.0, XB[:], AL.mult, AL.add)
            t5 = wpool.tile([128, T, K9], F32, tag="t5")
            STT(t5[:], PP[:], 2.0, QQ[:], AL.mult, AL.add)
            t6 = wpool.tile([128, T, K9], F32, tag="t6")
            STT(t6[:], IY[:], 64.0, IX[:], AL.mult, AL.add)
            idxf = wpool.tile([128, T, K9], F32, tag="idxf")
            STT(idxf[:], t5[:], 4096.0, t6[:], AL.mult, AL.add)
            nc.vector.tensor_copy(IDX[:], idxf[:])

            # W4 cells
            TT(_sap(W4[:], 0, [[K9 * 4, T], [4, K9]]), V0[:], U0[:], AL.mult)
            TT(_sap(W4[:], 1, [[K9 * 4, T], [4, K9]]), V0[:], U1[:], AL.mult)
            TT(_sap(W4[:], 2, [[K9 * 4, T], [4, K9]]), V1[:], U0[:], AL.mult)
            TT(_sap(W4[:], 3, [[K9 * 4, T], [4, K9]]), V1[:], U1[:], AL.mult)

            # ---- phase 4: gather + weight + transpose + conv ----
            for t in range(T):
                G = gpool.tile([128, K9, 4, C], F32, tag="g")
                for k in range(K9):
                    nc.gpsimd.indirect_dma_start(
                        G[:, k, :, :].rearrange("p a c -> p (a c)"), None, pbc[:],
                        bass.IndirectOffsetOnAxis(ap=IDX[:, t, k : k + 1], axis=0),
                    )
                prod = mpool.tile([128, K9, 4, C], F32, tag="prod")
                w4b = _sap(W4[:], t * K9 * 4, [[4, K9], [1, 4], [0, C]])
                TT(prod[:], G[:], w4b, AL.mult)
                cadd = mpool.tile([128, K9, 2, C], F32, tag="cadd")
                nc.any.tensor_tensor(cadd[:], prod[:, :, 0:2, :], prod[:, :, 2:4, :], AL.add)
                val = mpool.tile([128, K9, C], F32, tag="val")
                nc.any.tensor_tensor(val[:], cadd[:, :, 0, :], cadd[:, :, 1, :], AL.add)

                tp = psA.tile([C, K9, 128], F32, tag="big")
                for k in range(K9):
                    nc.tensor.matmul(tp[:, k, :], val[:, k, :], idt[:],
                                     is_transpose=True, start=True, stop=True)
                vch = mpool.tile([C, K9, 128], F32, tag="vch")
                nc.scalar.copy(vch[:], tp[:])

                po = psB.tile([O, 128], F32, tag="po")
                for k in range(K9):
                    nc.tensor.matmul(po[:], w2s[:, k, :], vch[:, k, :],
                                     start=(k == 0), stop=(k == K9 - 1))
                nc.scalar.copy(OPRE[:, t, :], po[:])
                nc.vector.tensor_reduce(ST1[:, t : t + 1], OPRE[:, t, :],
                                        mybir.AxisListType.X, AL.add)
                sq = mpool.tile([O, 128], F32, tag="sq")
                TT(sq[:], OPRE[:, t, :], OPRE[:, t, :], AL.mult)
                nc.vector.tensor_reduce(ST2[:, t : t + 1], sq[:],
                                        mybir.AxisListType.X, AL.add)

            # ---- phase 5: BN + ReLU ----
            s1 = ppool.tile([O, 2], F32, tag="s1")
            nc.vector.tensor_reduce(s1[:, 0:1], ST1[:], mybir.AxisListType.X, AL.add)
            nc.vector.tensor_reduce(s1[:, 1:2], ST2[:], mybir.AxisListType.X, AL.add)
            if collective:
                cin = dpool.tile([O, 2], F32, tag="cin")
                cout = dpool.tile([O, 2], F32, tag="cout")
                nc.sync.dma_start(cin[:], s1[:])
                nc.gpsimd.collective_compute(
                    "AllReduce", AL.add,
                    replica_groups=[list(range(NCORES))],
                    ins=[cin.opt()], outs=[cout.opt()],
                )
                sg = ppool.tile([O, 2], F32, tag="sg")
                nc.sync.dma_start(sg[:], cout[:])
                denom = float(NPIX_TOT)
            else:
                sg = s1
                denom = float(T * W)

            mean = ppool.tile([O, 1], F32, tag="mean")
            TS(mean[:], sg[:, 0:1], 1.0 / denom, None, AL.mult)
            ex2 = ppool.tile([O, 1], F32, tag="ex2")
            TS(ex2[:], sg[:, 1:2], 1.0 / denom, None, AL.mult)
            m2 = ppool.tile([O, 1], F32, tag="m2")
            TT(m2[:], mean[:], mean[:], AL.mult)
            var = ppool.tile([O, 1], F32, tag="var")
            TT(var[:], ex2[:], m2[:], AL.subtract)
            stdt = ppool.tile([O, 1], F32, tag="stdt")
            nc.scalar.activation(stdt[:], var[:],
                                 mybir.ActivationFunctionType.Sqrt,
                                 bias=epst[0:O, :])
            rstd = ppool.tile([O, 1], F32, tag="rstd")
            nc.vector.reciprocal(rstd[:], stdt[:])
            scl = ppool.tile([O, 1], F32, tag="scl")
            TT(scl[:], gbs[:, 0:1], rstd[:], AL.mult)
            msc = ppool.tile([O, 1], F32, tag="msc")
            TT(msc[:], mean[:], scl[:], AL.mult)
            sh = ppool.tile([O, 1], F32, tag="sh")
            TT(sh[:], gbs[:, 1:2], msc[:], AL.subtract)

            for t in range(T):
                nc.scalar.activation(ON[:, t, :], OPRE[:, t, :],
                                     mybir.ActivationFunctionType.Relu,
                                     bias=sh[:], scale=scl[:])
            nc.sync.dma_start(outd[:], ON[:])

    if fixup:
        fix_multiwait(nc)
    return nc


# ---------------- host-side preparation ----------------

def _host_prep(x, conv_w, off_w, off_b, mask_w, mask_b, gamma, beta, T=RPC):
    """Build the 8 per-core input maps."""
    x = np.asarray(x, np.float32)
    K9 = 9

    # patch buffer per batch: PB[b, 2p+q, i*64+j, :] = 2x2 patch at
    # rows (2i+p, 2i+p+1), cols (2j+q, 2j+q+1), channels-last, flattened.
    xcl = np.transpose(x, (0, 2, 3, 1))  # [B, H, W, C]
    xpad = np.zeros((B, H + 2, W + 2, C), np.float32)
    xpad[:, :H, :W] = xcl
    PB = np.zeros((B, 4, 4096, 256), np.float32)
    for p in range(2):
        for q in range(2):
            # patch (i, j): rows (2i+p, 2i+p+1), cols (2j+q, 2j+q+1)
            sub = xpad[:, p : p + 129, q : q + 129, :]  # rows p..p+128
            r0 = sub[:, 0:128:2, 0:128:2, :]   # (b, i, j, c) top-left
            r1 = sub[:, 0:128:2, 1:129:2, :]   # top-right
            r2 = sub[:, 1:129:2, 0:128:2, :]   # bottom-left
            r3 = sub[:, 1:129:2, 1:129:2, :]   # bottom-right
            patch = np.stack([r0, r1, r2, r3], axis=3)  # [B, 64, 64, 4, C]
            PB[:, 2 * p + q] = patch.reshape(B, 4096, 256)

    # wcat: [C+1, 9, 27]; channel rows = cat(off_w, mask_w) transposed; ones row = biases at k=0
    wfull = np.concatenate([off_w, mask_w], axis=0)  # [27, C, 3, 3]
    wcat = np.zeros((C + 1, K9, 27), np.float32)
    wcat[:C] = np.transpose(wfull.reshape(27, C, K9), (1, 2, 0))
    bias = np.concatenate([off_b, mask_b]).astype(np.float32)  # [27]
    wcat[C, 0, :] = bias

    w2 = np.transpose(conv_w.reshape(O, C, K9), (1, 2, 0)).astype(np.float32)  # [C, 9, O]
    ident = np.eye(128, dtype=np.float32)
    gb = np.stack([np.asarray(gamma, np.float32), np.asarray(beta, np.float32)], axis=1)

    ky = np.repeat(np.arange(3), 3).astype(np.float32)
    kx = np.tile(np.arange(3), 3).astype(np.float32)
    gx = np.arange(128, dtype=np.float32)

    in_maps = []
    for core in range(NCORES):
        b, strip = divmod(core, 4)
        r0 = strip * RPC
        # xslab [C+1, T+2, W+2]: rows r0-1 .. r0+T, zero padded, ones row
        xslab = np.zeros((C + 1, T + 2, W + 2), np.float32)
        lo, hi = r0 - 1, r0 + T + 1
        glo, ghi = max(lo, 0), min(hi, H)
        xslab[:C, (glo - lo) : (ghi - lo), 1 : W + 1] = x[b, :, glo:ghi, :]
        xslab[C] = 1.0
        cy = (r0 + np.arange(T)[None, :, None] + (ky - 1.0)[None, None, :]
              + np.zeros((128, 1, 1))).astype(np.float32)
        cx = (gx[:, None, None] + (kx - 1.0)[None, None, :]
              + np.zeros((1, T, 1))).astype(np.float32)
        in_maps.append({
            "xslab": xslab, "pbc": PB[b].reshape(4 * 4096, 256),
            "cy": cy, "cx": cx, "wcat": wcat, "w2": w2,
            "ident": ident, "gb": gb,
        })
    return in_maps


_NC_CACHE = {}


def kernel(x, conv_w, off_w, off_b, mask_w, mask_b, gamma, beta):
    if "nc" not in _NC_CACHE:
        _NC_CACHE["nc"] = build_nc()
    nc = _NC_CACHE["nc"]
    in_maps = _host_prep(x, conv_w, off_w, off_b, mask_w, mask_b, gamma, beta)
    res = run_bass_kernel_spmd(nc, in_maps, core_ids=list(range(NCORES)))
    out = np.zeros((B, O, H, W), np.float32)
    for core in range(NCORES):
        b, strip = divmod(core, 4)
        r0 = strip * RPC
        out[b, :, r0 : r0 + RPC, :] = res.results[core]["outn"]
    return out



# revision 29
# speedup vs baseline: 1.1461x; 1.1461x over previous
"""Deformable conv (DCNv2) + BN + ReLU Trainium2 Bass kernel, v4.

Sharding: 8 cores = (2 batches) x (4 H-strips of 32 rows). Per core:
  1. 3x3 offset/mask conv via bf16 PE matmuls (pixel-major output, bias
     via ones-channel trick), processed in half-strips so coords and
     gathers start early.
  2. Bilinear sample positions -> per-pixel patch index + 4 cell weights
     (fp32; floor = int-convert(x - 0.4999999), comparison-free). The
     patch grid is padded (positions -4..131) with zeros outside the
     image so no boundary masking is needed.
  3. One indirect DMA per 4-tile group gathers 2x2x64ch bf16 patches
     ([p][c][q] row layout).
  4. Bilinear cell weighting in-place (DVE bf16 2x TT, weights broadcast
     along c), y-cell fold (packed bf16 add), PE-transpose (c,q)-chunks
     to channel-major, main conv via bf16 PE matmuls with q-duplicated
     weights (the x-cell fold rides the contraction).
  5. BN partial sums via Act accumulators -> AllGather + local sum ->
     scale/shift + ReLU.

Host side stages per-core inputs (bf16 x-slab with halo + ones row,
padded parity patch buffer, weights) and reassembles the output.
"""

import numpy as np
import ml_dtypes
import concourse.bass as bass
import concourse.mybir as mybir
import concourse.tile as tile
from concourse.bass_utils import run_bass_kernel_spmd
from contextlib import ExitStack

F32 = mybir.dt.float32
BF16 = mybir.dt.bfloat16
I32 = mybir.dt.int32
BF = ml_dtypes.bfloat16

B, C, O, H, W = 2, 64, 64, 128, 128
NCORES = 8
RPC = H // 4            # rows per core (4 strips per batch)
NPIX_TOT = B * H * W    # BN denominator
BN_EPS = 1e-5
K9 = 9
GRID = 68               # padded parity grid: iy/ix in [0, 68)
PBROWS = 4 * GRID * GRID
GT = 2                  # row-tiles per gather group
FLOORB = -0.49999997    # floor(x) == round_to_int(x + FLOORB) up to eps
AF = mybir.ActivationFunctionType
AX = mybir.AxisListType


def _sap(ap, off_elems, dims):
    """AP with same tensor/partition dim, custom free dims."""
    return bass.AP(ap.tensor, ap.offset + off_elems, [ap.ap[0]] + dims)


def fix_multiwait(nc):
    """This env's walrus allows only ONE sem wait per instruction; split
    extras into single-wait drains on the same engine immediately before."""
    for f in nc.m.functions:
        for blk in f.blocks:
            i = 0
            while i < len(blk.instructions):
                ins = blk.instructions[i]
                si = ins.sync_info
                if si is not None and si.on_wait and len(si.on_wait) > 1:
                    waits = list(si.on_wait)
                    si.on_wait = [waits[-1]]
                    for j, w in enumerate(waits[:-1]):
                        d2 = mybir.InstDrain(
                            name=f"{ins.name}-wsplit{j}", ins=[], outs=[],
                            engine=ins.engine,
                        )
                        d2.sync_info = mybir.SyncInfo(on_wait=[w], on_update=[])
                        blk.instructions.insert(i, d2)
                        i += 1
                i += 1


def build_nc(T=RPC, collective=True, fixup=True):
    nc = bass.Bass(dynamic_dma_scratch_size=40960)
    NG = T // GT            # gather groups
    HT = T // 2             # tiles per half-strip

    # ---- per-core external inputs (host-staged) ----
    xslab = nc.dram_tensor("xslab", [C + 1, T + 2, W + 2], BF16, kind="ExternalInput")
    pbc = nc.dram_tensor("pbc", [PBROWS, 256], BF16, kind="ExternalInput")
    cy = nc.dram_tensor("cy", [128, T, K9], F32, kind="ExternalInput")
    cx = nc.dram_tensor("cx", [128, T, K9], F32, kind="ExternalInput")
    wcat = nc.dram_tensor("wcat", [C + 1, K9, 27], BF16, kind="ExternalInput")
    w2q = nc.dram_tensor("w2q", [128, K9, O], BF16, kind="ExternalInput")
    ident = nc.dram_tensor("ident", [128, 128], BF16, kind="ExternalInput")
    gb = nc.dram_tensor("gb", [O, 2], F32, kind="ExternalInput")
    outd = nc.dram_tensor("outn", [O, T, W], F32, kind="ExternalOutput")

    with tile.TileContext(nc) as tc:
        with ExitStack() as ctx:
            cpool = ctx.enter_context(tc.tile_pool(name="const", bufs=1))
            ppool = ctx.enter_context(tc.tile_pool(name="persist", bufs=1))
            wpool = ctx.enter_context(tc.tile_pool(name="wtmp", bufs=1))
            gpool = ctx.enter_context(tc.tile_pool(name="gath", bufs=2))
            mpool = ctx.enter_context(tc.tile_pool(name="mac", bufs=2))
            psP = ctx.enter_context(tc.tile_pool(name="psP", bufs=1, space="PSUM"))
            psT = ctx.enter_context(tc.tile_pool(name="psT", bufs=2, space="PSUM"))
            psB = ctx.enter_context(tc.tile_pool(name="psB", bufs=2, space="PSUM"))
            dpool = ctx.enter_context(tc.tile_pool(name="dram", bufs=1, space="DRAM"))

            TT = nc.vector.tensor_tensor
            TS = nc.vector.tensor_scalar
            STT = nc.vector.scalar_tensor_tensor
            PTT = nc.gpsimd.tensor_tensor
            PTS = nc.gpsimd.tensor_scalar
            AL = mybir.AluOpType

            # ---- prologue loads ----
            xs = cpool.tile([C + 1, T + 2, W + 2], BF16, tag="xs")
            nc.sync.dma_start(xs[:], xslab[:])
            wc = cpool.tile([C + 1, K9, 27], BF16, tag="wc")
            nc.sync.dma_start(wc[:], wcat[:])
            w2s = cpool.tile([128, K9, O], BF16, tag="w2s")
            nc.sync.dma_start(w2s[:], w2q[:])
            idt = cpool.tile([128, 128], BF16, tag="idt")
            nc.sync.dma_start(idt[:], ident[:])
            cys = cpool.tile([128, T, K9], F32, tag="cys")
            nc.sync.dma_start(cys[:], cy[:])
            cxs = cpool.tile([128, T, K9], F32, tag="cxs")
            nc.sync.dma_start(cxs[:], cx[:])
            gbs = cpool.tile([O, 2], F32, tag="gbs")
            nc.sync.dma_start(gbs[:], gb[:])
            epst = cpool.tile([128, 1], F32, tag="epst")
            nc.vector.memset(epst[:], BN_EPS)
            # PE p-state warm-up: ~3us of dummy matmuls so phase 1 runs at
            # full clock; also preload the Relu/Sqrt/Sigmoid/Square tables.
            wps = psP.tile([128, 128], F32, tag="warm", name="wps")
            for _ in range(16):
                nc.tensor.matmul(wps[:], idt[:], idt[:], start=True, stop=True)
            wact = cpool.tile([128, 1], F32, tag="wact")
            nc.scalar.activation(wact[:], epst[:], AF.Sigmoid)
            nc.scalar.activation(wact[:], epst[:], AF.Square)
            nc.scalar.activation(wact[:], epst[:], AF.Sqrt)
            nc.scalar.activation(wact[:], epst[:], AF.Relu)

            # ---- persistent tiles ----
            OFF = ppool.tile([128, T, 27], F32, tag="OFF")
            MK2 = ppool.tile([128, T, K9], F32, tag="MK2")
            W4 = ppool.tile([128, T, K9, 2, 2], BF16, tag="W4")
            IDX = ppool.tile([128, T, K9], I32, tag="IDX")
            ST1 = ppool.tile([O, T // 4], F32, tag="ST1")
            ST2 = ppool.tile([O, NG], F32, tag="ST2")
            OPRE = ppool.tile([O, T, W], F32, tag="OPRE")
            ON = ppool.tile([O, T, W], F32, tag="ON")

            def phase1_range(lo, n):
                """Offset/mask conv for tiles [lo, lo+n)."""
                for g4 in range(lo // 4, (lo + n) // 4):
                    pso = psP.tile([128, 4, 27], F32, tag="p1", name="pso", bufs=2)
                    for tt in range(4):
                        t = 4 * g4 + tt
                        for k in range(K9):
                            kyy, kxx = k // 3, k % 3
                            lhsT = _sap(xs[:], (t + kyy) * (W + 2) + kxx,
                                        [[1, 128]])
                            rhs = _sap(wc[:], k * 27, [[1, 27]])
                            nc.tensor.matmul(pso[:, tt, :], lhsT, rhs,
                                             start=(k == 0), stop=(k == K9 - 1))
                    nc.scalar.copy(OFF[:, 4 * g4 : 4 * g4 + 4, :], pso[:])
                nc.scalar.activation(
                    MK2[:, lo : lo + n, :],
                    _sap(OFF[:], lo * 27 + 18, [[27, n], [1, K9]]),
                    AF.Sigmoid)

            def coords_idx(lo, n):
                """Index chain (gates the gathers): floors, clamps, patch idx.
                y-chain on DVE, x-chain on Pool, floors on Act."""
                oy = lo * 27
                o3 = lo * K9
                dims = [[27, n], [2, K9]]
                d3 = [[K9, n], [1, K9]]

                def tf(tag):
                    return wpool.tile([128, T, K9], F32, tag=tag, name=tag)[
                        :, 0:n, :]

                def ti(tag):
                    return wpool.tile([128, T, K9], I32, tag=tag, name=tag)[
                        :, 0:n, :]

                def flor(dst, src, itmp, eng_copy):
                    """dst = floor(src): int round-trip + is_gt fix (works
                    under any hardware rounding mode)."""
                    nc.scalar.activation(itmp, src, AF.Copy)
                    eng_copy(dst, itmp)
                    g = wpool.tile([128, T, K9], F32, tag="flg", name="flg")[
                        :, 0:n, :]
                    TT(g, dst, src, AL.is_gt)
                    TT(dst, dst, g, AL.subtract)

                d = {}
                py = d["py"] = tf("py")
                TT(py, _sap(OFF[:], oy + 0, dims),
                   _sap(cys[:], o3, d3), AL.add)
                px = d["px"] = tf("px")
                PTT(px, _sap(OFF[:], oy + 1, dims),
                    _sap(cxs[:], o3, d3), AL.add)

                iy0 = ti("iy0")
                Y0 = d["Y0"] = tf("Y0")
                flor(Y0, py, iy0, nc.vector.tensor_copy)
                ix0 = ti("ix0")
                X0 = d["X0"] = tf("X0")
                flor(X0, px, ix0, nc.gpsimd.tensor_copy)

                YC = tf("YC")
                TS(YC, Y0, -4.0, 130.0, AL.max, AL.min)
                XC = tf("XC")
                PTS(XC, X0, -4.0, 130.0, AL.max, AL.min)
                uy = tf("uy")
                TS(uy, YC, 0.5, 2.0, AL.mult, AL.add)
                ux = tf("ux")
                PTS(ux, XC, 0.5, 2.0, AL.mult, AL.add)
                iyi = ti("iyi")
                IY = tf("IY")
                flor(IY, uy, iyi, nc.vector.tensor_copy)
                ixi = ti("ixi")
                IX = tf("IX")
                flor(IX, ux, ixi, nc.gpsimd.tensor_copy)
                my = tf("my")
                TT(my, uy, IY, AL.subtract)
                mx = tf("mx")
                PTT(mx, ux, IX, AL.subtract)

                cls2 = tf("cls2")
                STT(cls2, my, 2.0, mx, AL.mult, AL.add)
                bq = tf("bq")
                STT(bq, IY, float(GRID), IX, AL.mult, AL.add)
                idxf = tf("idxf")
                STT(idxf, cls2, float(2 * GRID * GRID), bq,
                    AL.mult, AL.add)
                nc.vector.tensor_copy(IDX[:, lo : lo + n, :], idxf)
                return d

            def coords_w4(lo, n, d):
                """Cell-weight chain; can overlap the first gathers."""
                o3 = lo * K9
                d3 = [[K9, n], [1, K9]]

                def tf(tag):
                    return wpool.tile([128, T, K9], F32, tag=tag, name=tag)[
                        :, 0:n, :]

                mkh = _sap(MK2[:], o3, d3)
                WY = tf("WY")
                TT(WY, d["py"], d["Y0"], AL.subtract)
                WX = tf("WX")
                PTT(WX, d["px"], d["X0"], AL.subtract)
                V1 = tf("V1")
                TT(V1, mkh, WY, AL.mult)
                V0 = tf("V0")
                TT(V0, mkh, V1, AL.subtract)

                # W4 cells [p][q] (bf16)
                w4d = [[4 * K9, n], [4, K9]]
                w01 = _sap(W4[:], lo * K9 * 4 + 1, w4d)
                w00 = _sap(W4[:], lo * K9 * 4 + 0, w4d)
                w11 = _sap(W4[:], lo * K9 * 4 + 3, w4d)
                w10 = _sap(W4[:], lo * K9 * 4 + 2, w4d)
                TT(w01, V0, WX, AL.mult)
                TT(w00, V0, w01, AL.subtract)
                PTT(w11, V1, WX, AL.mult)
                PTT(w10, V1, w11, AL.subtract)

            def gather_group(g):
                t0 = g * GT
                Gb = gpool.tile([128, GT, K9, 2, C, 2], BF16, tag="G",
                                name="Gb")
                # hardware consumes ONE index per partition per instruction
                for tt in range(GT):
                    for k in range(K9):
                        dst = _sap(Gb[:], (tt * K9 + k) * 2 * C * 2,
                                   [[1, 2 * C * 2]])
                        nc.gpsimd.indirect_dma_start(
                            dst, None, pbc[:],
                            bass.IndirectOffsetOnAxis(
                                ap=IDX[:, t0 + tt, k : k + 1], axis=0),
                        )
                return Gb

            pstate = {}

            def compute_group(g, Gb):
                t0 = g * GT
                po = pstate
                for tt in range(GT):
                    t = t0 + tt
                    base = tt * K9 * 2 * C * 2
                    # bilinear cell weighting (bf16 2x TT)
                    Gw = mpool.tile([128, K9, 2, C, 2], BF16, tag="Gw",
                                    name="Gw")
                    gview = _sap(Gb[:], base, [[2, 2 * K9 * C], [1, 2]])
                    w4b = _sap(W4[:], t * K9 * 4, [[2, 2 * K9], [0, C], [1, 2]])
                    TT(_sap(Gw[:], 0, [[2, 2 * K9 * C], [1, 2]]), gview, w4b,
                       AL.mult)
                    # y-cell fold (packed bf16 add), split DVE/Pool
                    Gp = mpool.tile([128, K9, 2 * C], BF16, tag="Gp", name="Gp")
                    TT(Gp[:],
                       _sap(Gw[:], 0, [[2 * C * 2, K9], [1, 2 * C]]),
                       _sap(Gw[:], 2 * C, [[2 * C * 2, K9], [1, 2 * C]]),
                       AL.add)
                    # transpose to channel-major (psum: 8+1 taps)
                    tpA = psT.tile([128, 8, 128], BF16, tag="tpA", name="tpA")
                    tpC = psT.tile([128, 128], BF16, tag="tpC", name="tpC", bufs=1)
                    for k in range(K9):
                        dstp = tpA[:, k, :] if k < 8 else tpC[:]
                        nc.tensor.matmul(dstp, Gp[:, k, :], idt[:],
                                         is_transpose=True,
                                         start=True, stop=True)
                    vch = mpool.tile([128, K9, 128], BF16, tag="vch",
                                     name="vch")
                    nc.scalar.copy(vch[:, 0:8, :], tpA[:])
                    nc.scalar.copy(vch[:, 8, :], tpC[:])
                    # main conv; 4 tiles per psum bank, copied out with ST1
                    # accumulation per pack
                    if t % 4 == 0:
                        po["b"] = psB.tile([O, 4, 128], F32, tag="po",
                                           name="po")
                    for k in range(K9):
                        nc.tensor.matmul(po["b"][:, t % 4, :], w2s[:, k, :],
                                         vch[:, k, :],
                                         start=(k == 0), stop=(k == K9 - 1))
                    if t % 4 == 3:
                        nc.scalar.activation(
                            OPRE[:, t - 3 : t + 1, :], po["b"], AF.Copy,
                            accum_out=ST1[:, t // 4 : t // 4 + 1])
                        sq = mpool.tile([O, 4, W], F32, tag="sq", name="sq")
                        nc.scalar.activation(
                            sq[:], OPRE[:, t - 3 : t + 1, :], AF.Square,
                            accum_out=ST2[:, t // 4 : t // 4 + 1])

            # ---- schedule: first quarter fast-path so gathers start early --
            phase1_range(0, 8)
            dq = coords_idx(0, 8)
            G0 = gather_group(0)
            G1 = gather_group(1)
            coords_w4(0, 8, dq)
            phase1_range(8, 24)
            d1 = coords_idx(8, 24)
            coords_w4(8, 24, d1)
            pend = [(0, G0), (1, G1)]
            for g in range(2, NG):
                pend.append((g, gather_group(g)))
                cg, cG = pend.pop(0)
                compute_group(cg, cG)
            for cg, cG in pend:
                compute_group(cg, cG)

            # ---- BN stats exchange + apply ----
            s1 = ppool.tile([O, 2], F32, tag="s1")
            nc.vector.tensor_reduce(s1[:, 0:1], ST1[:], AX.X, AL.add)
            nc.vector.tensor_reduce(s1[:, 1:2], ST2[:], AX.X, AL.add)
            if collective:
                cin = dpool.tile([O, 2], F32, tag="cin")
                cout = dpool.tile([NCORES * O, 2], F32, tag="cout")
                nc.sync.dma_start(cin[:], s1[:])
                nc.gpsimd.collective_compute(
                    "AllGather", AL.bypass,
                    replica_groups=[list(range(NCORES))],
                    ins=[cin.opt()], outs=[cout.opt()],
                )
                # load back as [O, 2, 8] (cores innermost) and reduce
                sg8 = ppool.tile([O, 2, NCORES], F32, tag="sg8")
                csrc = cout[:]
                src = bass.AP(csrc.tensor, csrc.offset,
                              [[2, O], [1, 2], [2 * O, NCORES]])
                nc.sync.dma_start(sg8[:], src)
                sg = ppool.tile([O, 2], F32, tag="sg")
                nc.vector.tensor_reduce(sg[:], sg8[:], AX.X, AL.add)
                denom = float(NPIX_TOT)
            else:
                sg = s1
                denom = float(T * W)

            mean = ppool.tile([O, 1], F32, tag="mean")
            TS(mean[:], sg[:, 0:1], 1.0 / denom, None, AL.mult)
            ex2 = ppool.tile([O, 1], F32, tag="ex2")
            TS(ex2[:], sg[:, 1:2], 1.0 / denom, None, AL.mult)
            m2 = ppool.tile([O, 1], F32, tag="m2")
            TT(m2[:], mean[:], mean[:], AL.mult)
            var = ppool.tile([O, 1], F32, tag="var")
            TT(var[:], ex2[:], m2[:], AL.subtract)
            stdt = ppool.tile([O, 1], F32, tag="stdt")
            nc.scalar.activation(stdt[:], var[:], AF.Sqrt, bias=epst[0:O, :])
            rstd = ppool.tile([O, 1], F32, tag="rstd")
            nc.vector.reciprocal(rstd[:], stdt[:])
            scl = ppool.tile([O, 1], F32, tag="scl")
            TT(scl[:], gbs[:, 0:1], rstd[:], AL.mult)
            msc = ppool.tile([O, 1], F32, tag="msc")
            TT(msc[:], mean[:], scl[:], AL.mult)
            sh = ppool.tile([O, 1], F32, tag="sh")
            TT(sh[:], gbs[:, 1:2], msc[:], AL.subtract)

            # BN apply + ReLU in 4 chunks, overlapped with output DMA
            CH = T // 4
            for cidx in range(4):
                tlo = cidx * CH
                nc.scalar.activation(ON[:, tlo : tlo + CH, :],
                                     OPRE[:, tlo : tlo + CH, :],
                                     AF.Relu, bias=sh[:], scale=scl[:])
                nc.sync.dma_start(outd[:, tlo : tlo + CH, :],
                                  ON[:, tlo : tlo + CH, :])

    if fixup:
        fix_multiwait(nc)
    return nc


# ---------------- host-side preparation ----------------

def _host_prep(x, conv_w, off_w, off_b, mask_w, mask_b, gamma, beta, T=RPC):
    """Build the 8 per-core input maps."""
    x = np.asarray(x, np.float32)

    # padded parity patch buffer, bf16, rows [p][c][q]
    xcl = np.transpose(x, (0, 2, 3, 1)).astype(BF)      # [B, H, W, C]
    xpad = np.zeros((B, 138, 138, C), BF)
    xpad[:, 4 : 4 + H, 4 : 4 + W] = xcl
    PB = np.zeros((B, 4, GRID, GRID, 2, C, 2), BF)
    for P in range(2):
        for Q in range(2):
            for p in range(2):
                for q in range(2):
                    sub = xpad[:, P + p : P + p + 2 * GRID : 2,
                               Q + q : Q + q + 2 * GRID : 2, :]
                    PB[:, 2 * P + Q, :, :, p, :, q] = sub
    PB = PB.reshape(B, PBROWS, 256)

    # wcat: [C+1, 9, 27]; channel rows = cat(off_w, mask_w) transposed;
    # ones row = biases at k=0
    wfull = np.concatenate([off_w, mask_w], axis=0)     # [27, C, 3, 3]
    wcat = np.zeros((C + 1, K9, 27), BF)
    wcat[:C] = np.transpose(wfull.reshape(27, C, K9), (1, 2, 0)).astype(BF)
    bias = np.concatenate([off_b, mask_b]).astype(BF)   # [27]
    wcat[C, 0, :] = bias

    # w2q: [128, 9, O] with partition j = c*2 + q -> w2[c, k, o] duplicated
    w2 = np.transpose(np.asarray(conv_w, np.float32).reshape(O, C, K9),
                      (1, 2, 0))                        # [C, 9, O]
    w2kco = np.transpose(w2, (1, 0, 2)).astype(BF)      # [9, C, O]
    w2q = np.zeros((128, K9, O), BF)
    w2q[0::2] = np.transpose(w2kco, (1, 0, 2))
    w2q[1::2] = np.transpose(w2kco, (1, 0, 2))

    ident = np.eye(128, dtype=np.float32).astype(BF)
    gb = np.stack([np.asarray(gamma, np.float32),
                   np.asarray(beta, np.float32)], axis=1)

    ky = np.repeat(np.arange(3), 3).astype(np.float32)
    kx = np.tile(np.arange(3), 3).astype(np.float32)
    gx = np.arange(128, dtype=np.float32)

    in_maps = []
    for core in range(NCORES):
        b, strip = divmod(core, 4)
        r0 = strip * RPC
        # xslab [C+1, T+2, W+2]: rows r0-1 .. r0+T, zero padded, ones row
        xslab = np.zeros((C + 1, T + 2, W + 2), BF)
        lo, hi = r0 - 1, r0 + T + 1
        glo, ghi = max(lo, 0), min(hi, H)
        xslab[:C, (glo - lo) : (ghi - lo), 1 : W + 1] = x[b, :, glo:ghi, :].astype(BF)
        xslab[C] = 1.0
        cyh = (r0 + np.arange(T)[None, :, None] + (ky - 1.0)[None, None, :]
               + np.zeros((128, 1, 1))).astype(np.float32)
        cxh = (gx[:, None, None] + (kx - 1.0)[None, None, :]
               + np.zeros((1, T, 1))).astype(np.float32)
        in_maps.append({
            "xslab": xslab, "pbc": PB[b],
            "cy": cyh, "cx": cxh, "wcat": wcat, "w2q": w2q,
            "ident": ident, "gb": gb,
        })
    return in_maps


_NC_CACHE = {}


def kernel(x, conv_w, off_w, off_b, mask_w, mask_b, gamma, beta):
    if "nc" not in _NC_CACHE:
        _NC_CACHE["nc"] = build_nc()
    nc = _NC_CACHE["nc"]
    in_maps = _host_prep(x, conv_w, off_w, off_b, mask_w, mask_b, gamma, beta)
    res = run_bass_kernel_spmd(nc, in_maps, core_ids=list(range(NCORES)))
    out = np.zeros((B, O, H, W), np.float32)
    for core in range(NCORES):
        b, strip = divmod(core, 4)
        r0 = strip * RPC
        out[b, :, r0 : r0 + RPC, :] = res.results[core]["outn"]
    return out


# revision 31
# speedup vs baseline: 1.1640x; 1.0157x over previous
"""Deformable conv (DCNv2) + BN + ReLU Trainium2 Bass kernel, v4.

Sharding: 8 cores = (2 batches) x (4 H-strips of 32 rows). Per core:
  1. 3x3 offset/mask conv via bf16 PE matmuls (pixel-major output, bias
     via ones-channel trick), processed in half-strips so coords and
     gathers start early.
  2. Bilinear sample positions -> per-pixel patch index + 4 cell weights
     (fp32; floor = int-convert(x - 0.4999999), comparison-free). The
     patch grid is padded (positions -4..131) with zeros outside the
     image so no boundary masking is needed.
  3. One indirect DMA per 4-tile group gathers 2x2x64ch bf16 patches
     ([p][c][q] row layout).
  4. Bilinear cell weighting in-place (DVE bf16 2x TT, weights broadcast
     along c), y-cell fold (packed bf16 add), PE-transpose (c,q)-chunks
     to channel-major, main conv via bf16 PE matmuls with q-duplicated
     weights (the x-cell fold rides the contraction).
  5. BN partial sums via Act accumulators -> AllGather + local sum ->
     scale/shift + ReLU.

Host side stages per-core inputs (bf16 x-slab with halo + ones row,
padded parity patch buffer, weights) and reassembles the output.
"""

import numpy as np
import ml_dtypes
import concourse.bass as bass
import concourse.mybir as mybir
import concourse.tile as tile
from concourse.bass_utils import run_bass_kernel_spmd
from contextlib import ExitStack

F32 = mybir.dt.float32
BF16 = mybir.dt.bfloat16
I32 = mybir.dt.int32
BF = ml_dtypes.bfloat16

B, C, O, H, W = 2, 64, 64, 128, 128
NCORES = 8
RPC = H // 4            # rows per core (4 strips per batch)
NPIX_TOT = B * H * W    # BN denominator
BN_EPS = 1e-5
K9 = 9
GRID = 68               # padded parity grid: iy/ix in [0, 68)
PBROWS = 4 * GRID * GRID
GT = 2                  # row-tiles per gather group
FLOORB = -0.49999997    # floor(x) == round_to_int(x + FLOORB) up to eps
AF = mybir.ActivationFunctionType
AX = mybir.AxisListType


def _sap(ap, off_elems, dims):
    """AP with same tensor/partition dim, custom free dims."""
    return bass.AP(ap.tensor, ap.offset + off_elems, [ap.ap[0]] + dims)


def fix_multiwait(nc):
    """This env's walrus allows only ONE sem wait per instruction; split
    extras into single-wait drains on the same engine immediately before."""
    for f in nc.m.functions:
        for blk in f.blocks:
            i = 0
            while i < len(blk.instructions):
                ins = blk.instructions[i]
                si = ins.sync_info
                if si is not None and si.on_wait and len(si.on_wait) > 1:
                    waits = list(si.on_wait)
                    si.on_wait = [waits[-1]]
                    for j, w in enumerate(waits[:-1]):
                        d2 = mybir.InstDrain(
                            name=f"{ins.name}-wsplit{j}", ins=[], outs=[],
                            engine=ins.engine,
                        )
                        d2.sync_info = mybir.SyncInfo(on_wait=[w], on_update=[])
                        blk.instructions.insert(i, d2)
                        i += 1
                i += 1


def build_nc(T=RPC, collective=True, fixup=True):
    nc = bass.Bass(dynamic_dma_scratch_size=40960)
    NG = T // GT            # gather groups
    HT = T // 2             # tiles per half-strip

    # ---- per-core external inputs (host-staged) ----
    xslab = nc.dram_tensor("xslab", [C + 1, T + 2, W + 2], BF16, kind="ExternalInput")
    pbc = nc.dram_tensor("pbc", [PBROWS, 256], BF16, kind="ExternalInput")
    cy = nc.dram_tensor("cy", [128, T, K9], F32, kind="ExternalInput")
    cx = nc.dram_tensor("cx", [128, T, K9], F32, kind="ExternalInput")
    wcat = nc.dram_tensor("wcat", [C + 1, K9, 27], BF16, kind="ExternalInput")
    w2q = nc.dram_tensor("w2q", [128, K9, O], BF16, kind="ExternalInput")
    ident = nc.dram_tensor("ident", [128, 128], BF16, kind="ExternalInput")
    gb = nc.dram_tensor("gb", [O, 2], F32, kind="ExternalInput")
    outd = nc.dram_tensor("outn", [O, T, W], F32, kind="ExternalOutput")

    with tile.TileContext(nc) as tc:
        with ExitStack() as ctx:
            cpool = ctx.enter_context(tc.tile_pool(name="const", bufs=1))
            ppool = ctx.enter_context(tc.tile_pool(name="persist", bufs=1))
            wpool = ctx.enter_context(tc.tile_pool(name="wtmp", bufs=1))
            gpool = ctx.enter_context(tc.tile_pool(name="gath", bufs=2))
            mpool = ctx.enter_context(tc.tile_pool(name="mac", bufs=2))
            psP = ctx.enter_context(tc.tile_pool(name="psP", bufs=1, space="PSUM"))
            psT = ctx.enter_context(tc.tile_pool(name="psT", bufs=2, space="PSUM"))
            psB = ctx.enter_context(tc.tile_pool(name="psB", bufs=2, space="PSUM"))
            dpool = ctx.enter_context(tc.tile_pool(name="dram", bufs=1, space="DRAM"))

            TT = nc.vector.tensor_tensor
            TS = nc.vector.tensor_scalar
            STT = nc.vector.scalar_tensor_tensor
            PTT = nc.gpsimd.tensor_tensor
            PTS = nc.gpsimd.tensor_scalar
            AL = mybir.AluOpType

            # ---- prologue loads ----
            xs = cpool.tile([C + 1, T + 2, W + 2], BF16, tag="xs")
            nc.sync.dma_start(xs[:], xslab[:])
            wc = cpool.tile([C + 1, K9, 27], BF16, tag="wc")
            nc.sync.dma_start(wc[:], wcat[:])
            w2s = cpool.tile([128, K9, O], BF16, tag="w2s")
            nc.sync.dma_start(w2s[:], w2q[:])
            idt = cpool.tile([128, 128], BF16, tag="idt")
            nc.sync.dma_start(idt[:], ident[:])
            cys = cpool.tile([128, T, K9], F32, tag="cys")
            nc.sync.dma_start(cys[:], cy[:])
            cxs = cpool.tile([128, T, K9], F32, tag="cxs")
            nc.sync.dma_start(cxs[:], cx[:])
            gbs = cpool.tile([O, 2], F32, tag="gbs")
            nc.sync.dma_start(gbs[:], gb[:])
            epst = cpool.tile([128, 1], F32, tag="epst")
            nc.vector.memset(epst[:], BN_EPS)
            # PE p-state warm-up: ~3us of dummy matmuls so phase 1 runs at
            # full clock; also preload the Relu/Sqrt/Sigmoid/Square tables.
            wps = psP.tile([128, 128], F32, tag="warm", name="wps")
            for _ in range(16):
                nc.tensor.matmul(wps[:], idt[:], idt[:], start=True, stop=True)
            wact = cpool.tile([128, 1], F32, tag="wact")
            nc.scalar.activation(wact[:], epst[:], AF.Sigmoid)
            nc.scalar.activation(wact[:], epst[:], AF.Square)
            nc.scalar.activation(wact[:], epst[:], AF.Sqrt)
            nc.scalar.activation(wact[:], epst[:], AF.Relu)

            # ---- persistent tiles ----
            OFF = ppool.tile([128, T, 27], F32, tag="OFF")
            MK2 = ppool.tile([128, T, K9], F32, tag="MK2")
            W4 = ppool.tile([128, T, K9, 2, 2], BF16, tag="W4")
            IDX = ppool.tile([128, T, K9], I32, tag="IDX")
            ST1 = ppool.tile([O, T // 4], F32, tag="ST1")
            ST2 = ppool.tile([O, NG], F32, tag="ST2")
            OPRE = ppool.tile([O, T, W], F32, tag="OPRE")
            ON = ppool.tile([O, T, W], F32, tag="ON")

            def phase1_range(lo, n):
                """Offset/mask conv for tiles [lo, lo+n)."""
                for g4 in range(lo // 4, (lo + n) // 4):
                    pso = psP.tile([128, 4, 27], F32, tag="p1", name="pso", bufs=2)
                    for tt in range(4):
                        t = 4 * g4 + tt
                        for k in range(K9):
                            kyy, kxx = k // 3, k % 3
                            lhsT = _sap(xs[:], (t + kyy) * (W + 2) + kxx,
                                        [[1, 128]])
                            rhs = _sap(wc[:], k * 27, [[1, 27]])
                            nc.tensor.matmul(pso[:, tt, :], lhsT, rhs,
                                             start=(k == 0), stop=(k == K9 - 1))
                    nc.scalar.copy(OFF[:, 4 * g4 : 4 * g4 + 4, :], pso[:])
                nc.scalar.activation(
                    MK2[:, lo : lo + n, :],
                    _sap(OFF[:], lo * 27 + 18, [[27, n], [1, K9]]),
                    AF.Sigmoid)

            def coords_idx(lo, n):
                """Index chain (gates the gathers): floors, clamps, patch idx.
                y-chain on DVE, x-chain on Pool, floors on Act."""
                oy = lo * 27
                o3 = lo * K9
                dims = [[27, n], [2, K9]]
                d3 = [[K9, n], [1, K9]]

                def tf(tag):
                    return wpool.tile([128, T, K9], F32, tag=tag, name=tag)[
                        :, 0:n, :]

                def ti(tag):
                    return wpool.tile([128, T, K9], I32, tag=tag, name=tag)[
                        :, 0:n, :]

                def flor(dst, src, itmp, eng_copy):
                    """dst = floor(src): int round-trip + is_gt fix (works
                    under any hardware rounding mode)."""
                    nc.scalar.activation(itmp, src, AF.Copy)
                    eng_copy(dst, itmp)
                    g = wpool.tile([128, T, K9], F32, tag="flg", name="flg")[
                        :, 0:n, :]
                    TT(g, dst, src, AL.is_gt)
                    TT(dst, dst, g, AL.subtract)

                d = {}
                py = d["py"] = tf("py")
                TT(py, _sap(OFF[:], oy + 0, dims),
                   _sap(cys[:], o3, d3), AL.add)
                px = d["px"] = tf("px")
                TT(px, _sap(OFF[:], oy + 1, dims),
                   _sap(cxs[:], o3, d3), AL.add)

                iy0 = ti("iy0")
                Y0 = d["Y0"] = tf("Y0")
                flor(Y0, py, iy0, nc.vector.tensor_copy)
                ix0 = ti("ix0")
                X0 = d["X0"] = tf("X0")
                flor(X0, px, ix0, nc.vector.tensor_copy)

                YC = tf("YC")
                TS(YC, Y0, -4.0, 130.0, AL.max, AL.min)
                XC = tf("XC")
                TS(XC, X0, -4.0, 130.0, AL.max, AL.min)
                uy = tf("uy")
                TS(uy, YC, 0.5, 2.0, AL.mult, AL.add)
                ux = tf("ux")
                TS(ux, XC, 0.5, 2.0, AL.mult, AL.add)
                iyi = ti("iyi")
                IY = tf("IY")
                flor(IY, uy, iyi, nc.vector.tensor_copy)
                ixi = ti("ixi")
                IX = tf("IX")
                flor(IX, ux, ixi, nc.vector.tensor_copy)
                my = tf("my")
                TT(my, uy, IY, AL.subtract)
                mx = tf("mx")
                TT(mx, ux, IX, AL.subtract)

                cls2 = tf("cls2")
                STT(cls2, my, 2.0, mx, AL.mult, AL.add)
                bq = tf("bq")
                STT(bq, IY, float(GRID), IX, AL.mult, AL.add)
                idxf = tf("idxf")
                STT(idxf, cls2, float(2 * GRID * GRID), bq,
                    AL.mult, AL.add)
                nc.vector.tensor_copy(IDX[:, lo : lo + n, :], idxf)
                return d

            def coords_w4(lo, n, d):
                """Cell-weight chain; can overlap the first gathers."""
                o3 = lo * K9
                d3 = [[K9, n], [1, K9]]

                def tf(tag):
                    return wpool.tile([128, T, K9], F32, tag=tag, name=tag)[
                        :, 0:n, :]

                mkh = _sap(MK2[:], o3, d3)
                WY = tf("WY")
                TT(WY, d["py"], d["Y0"], AL.subtract)
                WX = tf("WX")
                TT(WX, d["px"], d["X0"], AL.subtract)
                V1 = tf("V1")
                TT(V1, mkh, WY, AL.mult)
                V0 = tf("V0")
                TT(V0, mkh, V1, AL.subtract)

                # W4 cells [p][q] (bf16)
                w4d = [[4 * K9, n], [4, K9]]
                w01 = _sap(W4[:], lo * K9 * 4 + 1, w4d)
                w00 = _sap(W4[:], lo * K9 * 4 + 0, w4d)
                w11 = _sap(W4[:], lo * K9 * 4 + 3, w4d)
                w10 = _sap(W4[:], lo * K9 * 4 + 2, w4d)
                TT(w01, V0, WX, AL.mult)
                TT(w00, V0, w01, AL.subtract)
                TT(w11, V1, WX, AL.mult)
                TT(w10, V1, w11, AL.subtract)

            def gather_group(g):
                t0 = g * GT
                Gb = gpool.tile([128, GT, K9, 2, C, 2], BF16, tag="G",
                                name="Gb")
                # hardware consumes ONE index per partition per instruction
                for tt in range(GT):
                    for k in range(K9):
                        dst = _sap(Gb[:], (tt * K9 + k) * 2 * C * 2,
                                   [[1, 2 * C * 2]])
                        nc.gpsimd.indirect_dma_start(
                            dst, None, pbc[:],
                            bass.IndirectOffsetOnAxis(
                                ap=IDX[:, t0 + tt, k : k + 1], axis=0),
                        )
                return Gb

            pstate = {}

            def compute_group(g, Gb):
                t0 = g * GT
                po = pstate
                for tt in range(GT):
                    t = t0 + tt
                    base = tt * K9 * 2 * C * 2
                    # bilinear cell weighting (bf16 2x TT)
                    Gw = mpool.tile([128, K9, 2, C, 2], BF16, tag="Gw",
                                    name="Gw")
                    gview = _sap(Gb[:], base, [[2, 2 * K9 * C], [1, 2]])
                    w4b = _sap(W4[:], t * K9 * 4, [[2, 2 * K9], [0, C], [1, 2]])
                    TT(_sap(Gw[:], 0, [[2, 2 * K9 * C], [1, 2]]), gview, w4b,
                       AL.mult)
                    # y-cell fold (packed bf16 add), split DVE/Pool
                    Gp = mpool.tile([128, K9, 2 * C], BF16, tag="Gp", name="Gp")
                    TT(Gp[:],
                       _sap(Gw[:], 0, [[2 * C * 2, K9], [1, 2 * C]]),
                       _sap(Gw[:], 2 * C, [[2 * C * 2, K9], [1, 2 * C]]),
                       AL.add)
                    # transpose to channel-major (psum: 8+1 taps)
                    tpA = psT.tile([128, 8, 128], BF16, tag="tpA", name="tpA")
                    tpC = psT.tile([128, 128], BF16, tag="tpC", name="tpC", bufs=1)
                    for k in range(K9):
                        dstp = tpA[:, k, :] if k < 8 else tpC[:]
                        nc.tensor.matmul(dstp, Gp[:, k, :], idt[:],
                                         is_transpose=True,
                                         start=True, stop=True)
                    vch = mpool.tile([128, K9, 128], BF16, tag="vch",
                                     name="vch")
                    nc.scalar.copy(vch[:, 0:8, :], tpA[:])
                    nc.scalar.copy(vch[:, 8, :], tpC[:])
                    # main conv; 4 tiles per psum bank, copied out with ST1
                    # accumulation per pack
                    if t % 4 == 0:
                        po["b"] = psB.tile([O, 4, 128], F32, tag="po",
                                           name="po")
                    for k in range(K9):
                        nc.tensor.matmul(po["b"][:, t % 4, :], w2s[:, k, :],
                                         vch[:, k, :],
                                         start=(k == 0), stop=(k == K9 - 1))
                    if t % 4 == 3:
                        nc.scalar.activation(
                            OPRE[:, t - 3 : t + 1, :], po["b"], AF.Copy,
                            accum_out=ST1[:, t // 4 : t // 4 + 1])
                        sq = mpool.tile([O, 4, W], F32, tag="sq", name="sq")
                        nc.scalar.activation(
                            sq[:], OPRE[:, t - 3 : t + 1, :], AF.Square,
                            accum_out=ST2[:, t // 4 : t // 4 + 1])

            # ---- schedule: first quarter fast-path so gathers start early --
            phase1_range(0, 4)
            dq = coords_idx(0, 4)
            G0 = gather_group(0)
            G1 = gather_group(1)
            coords_w4(0, 4, dq)
            phase1_range(4, 28)
            d1 = coords_idx(4, 28)
            coords_w4(4, 28, d1)
            pend = [(0, G0), (1, G1)]
            for g in range(2, NG):
                pend.append((g, gather_group(g)))
                cg, cG = pend.pop(0)
                compute_group(cg, cG)
            for cg, cG in pend:
                compute_group(cg, cG)

            # ---- BN stats exchange + apply ----
            s1 = ppool.tile([O, 2], F32, tag="s1")
            nc.vector.tensor_reduce(s1[:, 0:1], ST1[:], AX.X, AL.add)
            nc.vector.tensor_reduce(s1[:, 1:2], ST2[:], AX.X, AL.add)
            if collective:
                cin = dpool.tile([O, 2], F32, tag="cin")
                cout = dpool.tile([NCORES * O, 2], F32, tag="cout")
                nc.sync.dma_start(cin[:], s1[:])
                nc.gpsimd.collective_compute(
                    "AllGather", AL.bypass,
                    replica_groups=[list(range(NCORES))],
                    ins=[cin.opt()], outs=[cout.opt()],
                )
                # load back as [O, 2, 8] (cores innermost) and reduce
                sg8 = ppool.tile([O, 2, NCORES], F32, tag="sg8")
                csrc = cout[:]
                src = bass.AP(csrc.tensor, csrc.offset,
                              [[2, O], [1, 2], [2 * O, NCORES]])
                nc.sync.dma_start(sg8[:], src)
                sg = ppool.tile([O, 2], F32, tag="sg")
                nc.vector.tensor_reduce(sg[:], sg8[:], AX.X, AL.add)
                denom = float(NPIX_TOT)
            else:
                sg = s1
                denom = float(T * W)

            mean = ppool.tile([O, 1], F32, tag="mean")
            TS(mean[:], sg[:, 0:1], 1.0 / denom, None, AL.mult)
            ex2 = ppool.tile([O, 1], F32, tag="ex2")
            TS(ex2[:], sg[:, 1:2], 1.0 / denom, None, AL.mult)
            m2 = ppool.tile([O, 1], F32, tag="m2")
            TT(m2[:], mean[:], mean[:], AL.mult)
            var = ppool.tile([O, 1], F32, tag="var")
            TT(var[:], ex2[:], m2[:], AL.subtract)
            stdt = ppool.tile([O, 1], F32, tag="stdt")
            nc.scalar.activation(stdt[:], var[:], AF.Sqrt, bias=epst[0:O, :])
            rstd = ppool.tile([O, 1], F32, tag="rstd")
            nc.vector.reciprocal(rstd[:], stdt[:])
            scl = ppool.tile([O, 1], F32, tag="scl")
            TT(scl[:], gbs[:, 0:1], rstd[:], AL.mult)
            msc = ppool.tile([O, 1], F32, tag="msc")
            TT(msc[:], mean[:], scl[:], AL.mult)
            sh = ppool.tile([O, 1], F32, tag="sh")
            TT(sh[:], gbs[:, 1:2], msc[:], AL.subtract)

            # BN apply + ReLU in 4 chunks, overlapped with output DMA
            CH = T // 4
            for cidx in range(4):
                tlo = cidx * CH
                nc.scalar.activation(ON[:, tlo : tlo + CH, :],
                                     OPRE[:, tlo : tlo + CH, :],
                                     AF.Relu, bias=sh[:], scale=scl[:])
                nc.sync.dma_start(outd[:, tlo : tlo + CH, :],
                                  ON[:, tlo : tlo + CH, :])

    if fixup:
        fix_multiwait(nc)
    return nc


# ---------------- host-side preparation ----------------

def _host_prep(x, conv_w, off_w, off_b, mask_w, mask_b, gamma, beta, T=RPC):
    """Build the 8 per-core input maps."""
    x = np.asarray(x, np.float32)

    # padded parity patch buffer, bf16, rows [p][c][q]
    xcl = np.transpose(x, (0, 2, 3, 1)).astype(BF)      # [B, H, W, C]
    xpad = np.zeros((B, 138, 138, C), BF)
    xpad[:, 4 : 4 + H, 4 : 4 + W] = xcl
    PB = np.zeros((B, 4, GRID, GRID, 2, C, 2), BF)
    for P in range(2):
        for Q in range(2):
            for p in range(2):
                for q in range(2):
                    sub = xpad[:, P + p : P + p + 2 * GRID : 2,
                               Q + q : Q + q + 2 * GRID : 2, :]
                    PB[:, 2 * P + Q, :, :, p, :, q] = sub
    PB = PB.reshape(B, PBROWS, 256)

    # wcat: [C+1, 9, 27]; channel rows = cat(off_w, mask_w) transposed;
    # ones row = biases at k=0
    wfull = np.concatenate([off_w, mask_w], axis=0)     # [27, C, 3, 3]
    wcat = np.zeros((C + 1, K9, 27), BF)
    wcat[:C] = np.transpose(wfull.reshape(27, C, K9), (1, 2, 0)).astype(BF)
    bias = np.concatenate([off_b, mask_b]).astype(BF)   # [27]
    wcat[C, 0, :] = bias

    # w2q: [128, 9, O] with partition j = c*2 + q -> w2[c, k, o] duplicated
    w2 = np.transpose(np.asarray(conv_w, np.float32).reshape(O, C, K9),
                      (1, 2, 0))                        # [C, 9, O]
    w2kco = np.transpose(w2, (1, 0, 2)).astype(BF)      # [9, C, O]
    w2q = np.zeros((128, K9, O), BF)
    w2q[0::2] = np.transpose(w2kco, (1, 0, 2))
    w2q[1::2] = np.transpose(w2kco, (1, 0, 2))

    ident = np.eye(128, dtype=np.float32).astype(BF)
    gb = np.stack([np.asarray(gamma, np.float32),
                   np.asarray(beta, np.float32)], axis=1)

    ky = np.repeat(np.arange(3), 3).astype(np.float32)
    kx = np.tile(np.arange(3), 3).astype(np.float32)
    gx = np.arange(128, dtype=np.float32)

    in_maps = []
    for core in range(NCORES):
        b, strip = divmod(core, 4)
        r0 = strip * RPC
        # xslab [C+1, T+2, W+2]: rows r0-1 .. r0+T, zero padded, ones row
        xslab = np.zeros((C + 1, T + 2, W + 2), BF)
        lo, hi = r0 - 1, r0 + T + 1
        glo, ghi = max(lo, 0), min(hi, H)
        xslab[:C, (glo - lo) : (ghi - lo), 1 : W + 1] = x[b, :, glo:ghi, :].astype(BF)
        xslab[C] = 1.0
        cyh = (r0 + np.arange(T)[None, :, None] + (ky - 1.0)[None, None, :]
               + np.zeros((128, 1, 1))).astype(np.float32)
        cxh = (gx[:, None, None] + (kx - 1.0)[None, None, :]
               + np.zeros((1, T, 1))).astype(np.float32)
        in_maps.append({
            "xslab": xslab, "pbc": PB[b],
            "cy": cyh, "cx": cxh, "wcat": wcat, "w2q": w2q,
            "ident": ident, "gb": gb,
        })
    return in_maps


_NC_CACHE = {}


def kernel(x, conv_w, off_w, off_b, mask_w, mask_b, gamma, beta):
    if "nc" not in _NC_CACHE:
        _NC_CACHE["nc"] = build_nc()
    nc = _NC_CACHE["nc"]
    in_maps = _host_prep(x, conv_w, off_w, off_b, mask_w, mask_b, gamma, beta)
    res = run_bass_kernel_spmd(nc, in_maps, core_ids=list(range(NCORES)))
    out = np.zeros((B, O, H, W), np.float32)
    for core in range(NCORES):
        b, strip = divmod(core, 4)
        r0 = strip * RPC
        out[b, :, r0 : r0 + RPC, :] = res.results[core]["outn"]
    return out


# revision 32
# speedup vs baseline: 1.1641x; 1.0000x over previous
"""Deformable conv (DCNv2) + BN + ReLU Trainium2 Bass kernel, v4.

Sharding: 8 cores = (2 batches) x (4 H-strips of 32 rows). Per core:
  1. 3x3 offset/mask conv via bf16 PE matmuls (pixel-major output, bias
     via ones-channel trick), processed in half-strips so coords and
     gathers start early.
  2. Bilinear sample positions -> per-pixel patch index + 4 cell weights
     (fp32; floor = int-convert(x - 0.4999999), comparison-free). The
     patch grid is padded (positions -4..131) with zeros outside the
     image so no boundary masking is needed.
  3. One indirect DMA per 4-tile group gathers 2x2x64ch bf16 patches
     ([p][c][q] row layout).
  4. Bilinear cell weighting in-place (DVE bf16 2x TT, weights broadcast
     along c), y-cell fold (packed bf16 add), PE-transpose (c,q)-chunks
     to channel-major, main conv via bf16 PE matmuls with q-duplicated
     weights (the x-cell fold rides the contraction).
  5. BN partial sums via Act accumulators -> AllGather + local sum ->
     scale/shift + ReLU.

Host side stages per-core inputs (bf16 x-slab with halo + ones row,
padded parity patch buffer, weights) and reassembles the output.
"""

import numpy as np
import ml_dtypes
import concourse.bass as bass
import concourse.mybir as mybir
import concourse.tile as tile
from concourse.bass_utils import run_bass_kernel_spmd
from contextlib import ExitStack

F32 = mybir.dt.float32
BF16 = mybir.dt.bfloat16
I32 = mybir.dt.int32
BF = ml_dtypes.bfloat16

B, C, O, H, W = 2, 64, 64, 128, 128
NCORES = 8
RPC = H // 4            # rows per core (4 strips per batch)
NPIX_TOT = B * H * W    # BN denominator
BN_EPS = 1e-5
K9 = 9
GRID = 68               # padded parity grid: iy/ix in [0, 68)
PBROWS = 4 * GRID * GRID
GT = 2                  # row-tiles per gather group
FLOORB = -0.49999997    # floor(x) == round_to_int(x + FLOORB) up to eps
AF = mybir.ActivationFunctionType
AX = mybir.AxisListType


def _sap(ap, off_elems, dims):
    """AP with same tensor/partition dim, custom free dims."""
    return bass.AP(ap.tensor, ap.offset + off_elems, [ap.ap[0]] + dims)


def fix_multiwait(nc):
    """This env's walrus allows only ONE sem wait per instruction; split
    extras into single-wait drains on the same engine immediately before."""
    for f in nc.m.functions:
        for blk in f.blocks:
            i = 0
            while i < len(blk.instructions):
                ins = blk.instructions[i]
                si = ins.sync_info
                if si is not None and si.on_wait and len(si.on_wait) > 1:
                    waits = list(si.on_wait)
                    si.on_wait = [waits[-1]]
                    for j, w in enumerate(waits[:-1]):
                        d2 = mybir.InstDrain(
                            name=f"{ins.name}-wsplit{j}", ins=[], outs=[],
                            engine=ins.engine,
                        )
                        d2.sync_info = mybir.SyncInfo(on_wait=[w], on_update=[])
                        blk.instructions.insert(i, d2)
                        i += 1
                i += 1


def build_nc(T=RPC, collective=True, fixup=True):
    nc = bass.Bass(dynamic_dma_scratch_size=40960)
    NG = T // GT            # gather groups
    HT = T // 2             # tiles per half-strip

    # ---- per-core external inputs (host-staged) ----
    xslab = nc.dram_tensor("xslab", [C + 1, T + 2, W + 2], BF16, kind="ExternalInput")
    pbc = nc.dram_tensor("pbc", [PBROWS, 256], BF16, kind="ExternalInput")
    cy = nc.dram_tensor("cy", [128, T, K9], F32, kind="ExternalInput")
    cx = nc.dram_tensor("cx", [128, T, K9], F32, kind="ExternalInput")
    wcat = nc.dram_tensor("wcat", [C + 1, K9, 27], BF16, kind="ExternalInput")
    w2q = nc.dram_tensor("w2q", [128, K9, O], BF16, kind="ExternalInput")
    ident = nc.dram_tensor("ident", [128, 128], BF16, kind="ExternalInput")
    gb = nc.dram_tensor("gb", [O, 2], F32, kind="ExternalInput")
    outd = nc.dram_tensor("outn", [O, T, W], F32, kind="ExternalOutput")

    with tile.TileContext(nc) as tc:
        with ExitStack() as ctx:
            cpool = ctx.enter_context(tc.tile_pool(name="const", bufs=1))
            ppool = ctx.enter_context(tc.tile_pool(name="persist", bufs=1))
            wpool = ctx.enter_context(tc.tile_pool(name="wtmp", bufs=1))
            gpool = ctx.enter_context(tc.tile_pool(name="gath", bufs=2))
            mpool = ctx.enter_context(tc.tile_pool(name="mac", bufs=2))
            psP = ctx.enter_context(tc.tile_pool(name="psP", bufs=1, space="PSUM"))
            psT = ctx.enter_context(tc.tile_pool(name="psT", bufs=2, space="PSUM"))
            psB = ctx.enter_context(tc.tile_pool(name="psB", bufs=2, space="PSUM"))
            dpool = ctx.enter_context(tc.tile_pool(name="dram", bufs=1, space="DRAM"))

            TT = nc.vector.tensor_tensor
            TS = nc.vector.tensor_scalar
            STT = nc.vector.scalar_tensor_tensor
            PTT = nc.gpsimd.tensor_tensor
            PTS = nc.gpsimd.tensor_scalar
            AL = mybir.AluOpType

            # ---- prologue loads ----
            xs = cpool.tile([C + 1, T + 2, W + 2], BF16, tag="xs")
            nc.sync.dma_start(xs[:], xslab[:])
            wc = cpool.tile([C + 1, K9, 27], BF16, tag="wc")
            nc.sync.dma_start(wc[:], wcat[:])
            w2s = cpool.tile([128, K9, O], BF16, tag="w2s")
            nc.sync.dma_start(w2s[:], w2q[:])
            idt = cpool.tile([128, 128], BF16, tag="idt")
            nc.sync.dma_start(idt[:], ident[:])
            cys = cpool.tile([128, T, K9], F32, tag="cys")
            nc.sync.dma_start(cys[:], cy[:])
            cxs = cpool.tile([128, T, K9], F32, tag="cxs")
            nc.sync.dma_start(cxs[:], cx[:])
            gbs = cpool.tile([O, 2], F32, tag="gbs")
            nc.sync.dma_start(gbs[:], gb[:])
            epst = cpool.tile([128, 1], F32, tag="epst")
            nc.vector.memset(epst[:], BN_EPS)
            # PE p-state warm-up: ~3us of dummy matmuls so phase 1 runs at
            # full clock; also preload the Relu/Sqrt/Sigmoid/Square tables.
            wps = psP.tile([128, 128], F32, tag="warm", name="wps")
            for _ in range(16):
                nc.tensor.matmul(wps[:], idt[:], idt[:], start=True, stop=True)
            wact = cpool.tile([128, 1], F32, tag="wact")
            nc.scalar.activation(wact[:], epst[:], AF.Sigmoid)
            nc.scalar.activation(wact[:], epst[:], AF.Square)
            nc.scalar.activation(wact[:], epst[:], AF.Sqrt)
            nc.scalar.activation(wact[:], epst[:], AF.Relu)

            # ---- persistent tiles ----
            OFF = ppool.tile([128, T, 27], F32, tag="OFF")
            MK2 = ppool.tile([128, T, K9], F32, tag="MK2")
            W4 = ppool.tile([128, T, K9, 2, 2], BF16, tag="W4")
            IDX = ppool.tile([128, T, K9], I32, tag="IDX")
            ST1 = ppool.tile([O, T // 4], F32, tag="ST1")
            ST2 = ppool.tile([O, T // 4], F32, tag="ST2")
            OPRE = ppool.tile([O, T, W], F32, tag="OPRE")
            ON = ppool.tile([O, T, W], F32, tag="ON")

            def phase1_range(lo, n):
                """Offset/mask conv for tiles [lo, lo+n)."""
                for g4 in range(lo // 4, (lo + n) // 4):
                    pso = psP.tile([128, 4, 27], F32, tag="p1", name="pso", bufs=2)
                    for tt in range(4):
                        t = 4 * g4 + tt
                        for k in range(K9):
                            kyy, kxx = k // 3, k % 3
                            lhsT = _sap(xs[:], (t + kyy) * (W + 2) + kxx,
                                        [[1, 128]])
                            rhs = _sap(wc[:], k * 27, [[1, 27]])
                            nc.tensor.matmul(pso[:, tt, :], lhsT, rhs,
                                             start=(k == 0), stop=(k == K9 - 1))
                    nc.scalar.copy(OFF[:, 4 * g4 : 4 * g4 + 4, :], pso[:])
                nc.scalar.activation(
                    MK2[:, lo : lo + n, :],
                    _sap(OFF[:], lo * 27 + 18, [[27, n], [1, K9]]),
                    AF.Sigmoid)

            def coords_idx(lo, n):
                """Index chain (gates the gathers): floors, clamps, patch idx.
                y-chain on DVE, x-chain on Pool, floors on Act."""
                oy = lo * 27
                o3 = lo * K9
                dims = [[27, n], [2, K9]]
                d3 = [[K9, n], [1, K9]]

                def tf(tag):
                    return wpool.tile([128, T, K9], F32, tag=tag, name=tag)[
                        :, 0:n, :]

                def ti(tag):
                    return wpool.tile([128, T, K9], I32, tag=tag, name=tag)[
                        :, 0:n, :]

                def flor(dst, src, itmp, eng_copy):
                    """dst = floor(src): int round-trip + is_gt fix (works
                    under any hardware rounding mode)."""
                    nc.scalar.activation(itmp, src, AF.Copy)
                    eng_copy(dst, itmp)
                    g = wpool.tile([128, T, K9], F32, tag="flg", name="flg")[
                        :, 0:n, :]
                    TT(g, dst, src, AL.is_gt)
                    TT(dst, dst, g, AL.subtract)

                d = {}
                py = d["py"] = tf("py")
                TT(py, _sap(OFF[:], oy + 0, dims),
                   _sap(cys[:], o3, d3), AL.add)
                px = d["px"] = tf("px")
                TT(px, _sap(OFF[:], oy + 1, dims),
                   _sap(cxs[:], o3, d3), AL.add)

                iy0 = ti("iy0")
                Y0 = d["Y0"] = tf("Y0")
                flor(Y0, py, iy0, nc.vector.tensor_copy)
                ix0 = ti("ix0")
                X0 = d["X0"] = tf("X0")
                flor(X0, px, ix0, nc.vector.tensor_copy)

                YC = tf("YC")
                TS(YC, Y0, -4.0, 130.0, AL.max, AL.min)
                XC = tf("XC")
                TS(XC, X0, -4.0, 130.0, AL.max, AL.min)
                uy = tf("uy")
                TS(uy, YC, 0.5, 2.0, AL.mult, AL.add)
                ux = tf("ux")
                TS(ux, XC, 0.5, 2.0, AL.mult, AL.add)
                iyi = ti("iyi")
                IY = tf("IY")
                flor(IY, uy, iyi, nc.vector.tensor_copy)
                ixi = ti("ixi")
                IX = tf("IX")
                flor(IX, ux, ixi, nc.vector.tensor_copy)
                my = tf("my")
                TT(my, uy, IY, AL.subtract)
                mx = tf("mx")
                TT(mx, ux, IX, AL.subtract)

                cls2 = tf("cls2")
                STT(cls2, my, 2.0, mx, AL.mult, AL.add)
                bq = tf("bq")
                STT(bq, IY, float(GRID), IX, AL.mult, AL.add)
                idxf = tf("idxf")
                STT(idxf, cls2, float(2 * GRID * GRID), bq,
                    AL.mult, AL.add)
                nc.vector.tensor_copy(IDX[:, lo : lo + n, :], idxf)
                return d

            def coords_w4(lo, n, d):
                """Cell-weight chain; can overlap the first gathers."""
                o3 = lo * K9
                d3 = [[K9, n], [1, K9]]

                def tf(tag):
                    return wpool.tile([128, T, K9], F32, tag=tag, name=tag)[
                        :, 0:n, :]

                mkh = _sap(MK2[:], o3, d3)
                WY = tf("WY")
                TT(WY, d["py"], d["Y0"], AL.subtract)
                WX = tf("WX")
                TT(WX, d["px"], d["X0"], AL.subtract)
                V1 = tf("V1")
                TT(V1, mkh, WY, AL.mult)
                V0 = tf("V0")
                TT(V0, mkh, V1, AL.subtract)

                # W4 cells [p][q] (bf16)
                w4d = [[4 * K9, n], [4, K9]]
                w01 = _sap(W4[:], lo * K9 * 4 + 1, w4d)
                w00 = _sap(W4[:], lo * K9 * 4 + 0, w4d)
                w11 = _sap(W4[:], lo * K9 * 4 + 3, w4d)
                w10 = _sap(W4[:], lo * K9 * 4 + 2, w4d)
                TT(w01, V0, WX, AL.mult)
                TT(w00, V0, w01, AL.subtract)
                TT(w11, V1, WX, AL.mult)
                TT(w10, V1, w11, AL.subtract)

            def gather_group(g):
                t0 = g * GT
                Gb = gpool.tile([128, GT, K9, 2, C, 2], BF16, tag="G",
                                name="Gb")
                # hardware consumes ONE index per partition per instruction
                for tt in range(GT):
                    for k in range(K9):
                        dst = _sap(Gb[:], (tt * K9 + k) * 2 * C * 2,
                                   [[1, 2 * C * 2]])
                        nc.gpsimd.indirect_dma_start(
                            dst, None, pbc[:],
                            bass.IndirectOffsetOnAxis(
                                ap=IDX[:, t0 + tt, k : k + 1], axis=0),
                        )
                return Gb

            pstate = {}

            def compute_group(g, Gb):
                t0 = g * GT
                po = pstate
                for tt in range(GT):
                    t = t0 + tt
                    base = tt * K9 * 2 * C * 2
                    # bilinear cell weighting (bf16 2x TT)
                    Gw = mpool.tile([128, K9, 2, C, 2], BF16, tag="Gw",
                                    name="Gw")
                    gview = _sap(Gb[:], base, [[2, 2 * K9 * C], [1, 2]])
                    w4b = _sap(W4[:], t * K9 * 4, [[2, 2 * K9], [0, C], [1, 2]])
                    TT(_sap(Gw[:], 0, [[2, 2 * K9 * C], [1, 2]]), gview, w4b,
                       AL.mult)
                    # y-cell fold (packed bf16 add), split DVE/Pool
                    Gp = mpool.tile([128, K9, 2 * C], BF16, tag="Gp", name="Gp")
                    TT(Gp[:],
                       _sap(Gw[:], 0, [[2 * C * 2, K9], [1, 2 * C]]),
                       _sap(Gw[:], 2 * C, [[2 * C * 2, K9], [1, 2 * C]]),
                       AL.add)
                    # transpose to channel-major (psum: 8+1 taps)
                    tpA = psT.tile([128, 8, 128], BF16, tag="tpA", name="tpA")
                    tpC = psT.tile([128, 128], BF16, tag="tpC", name="tpC", bufs=1)
                    for k in range(K9):
                        dstp = tpA[:, k, :] if k < 8 else tpC[:]
                        nc.tensor.matmul(dstp, Gp[:, k, :], idt[:],
                                         is_transpose=True,
                                         start=True, stop=True)
                    vch = mpool.tile([128, K9, 128], BF16, tag="vch",
                                     name="vch")
                    nc.scalar.copy(vch[:, 0:8, :], tpA[:])
                    nc.scalar.copy(vch[:, 8, :], tpC[:])
                    # main conv; 4 tiles per psum bank, copied out with ST1
                    # accumulation per pack
                    if t % 4 == 0:
                        po["b"] = psB.tile([O, 4, 128], F32, tag="po",
                                           name="po")
                    for k in range(K9):
                        nc.tensor.matmul(po["b"][:, t % 4, :], w2s[:, k, :],
                                         vch[:, k, :],
                                         start=(k == 0), stop=(k == K9 - 1))
                    if t % 4 == 3:
                        nc.scalar.activation(
                            OPRE[:, t - 3 : t + 1, :], po["b"], AF.Copy,
                            accum_out=ST1[:, t // 4 : t // 4 + 1])
                        sq = mpool.tile([O, 4, W], F32, tag="sq", name="sq")
                        nc.scalar.activation(
                            sq[:], OPRE[:, t - 3 : t + 1, :], AF.Square,
                            accum_out=ST2[:, t // 4 : t // 4 + 1])

            # ---- schedule: first quarter fast-path so gathers start early --
            phase1_range(0, 4)
            dq = coords_idx(0, 4)
            G0 = gather_group(0)
            G1 = gather_group(1)
            coords_w4(0, 4, dq)
            phase1_range(4, 28)
            d1 = coords_idx(4, 28)
            coords_w4(4, 28, d1)
            pend = [(0, G0), (1, G1)]
            for g in range(2, NG):
                pend.append((g, gather_group(g)))
                cg, cG = pend.pop(0)
                compute_group(cg, cG)
            for cg, cG in pend:
                compute_group(cg, cG)

            # ---- BN stats exchange + apply ----
            s1 = ppool.tile([O, 2], F32, tag="s1")
            nc.vector.tensor_reduce(s1[:, 0:1], ST1[:], AX.X, AL.add)
            nc.vector.tensor_reduce(s1[:, 1:2], ST2[:], AX.X, AL.add)
            if collective:
                cin = dpool.tile([O, 2], F32, tag="cin")
                cout = dpool.tile([NCORES * O, 2], F32, tag="cout")
                nc.sync.dma_start(cin[:], s1[:])
                nc.gpsimd.collective_compute(
                    "AllGather", AL.bypass,
                    replica_groups=[list(range(NCORES))],
                    ins=[cin.opt()], outs=[cout.opt()],
                )
                # load back as [O, 2, 8] (cores innermost) and reduce
                sg8 = ppool.tile([O, 2, NCORES], F32, tag="sg8")
                csrc = cout[:]
                src = bass.AP(csrc.tensor, csrc.offset,
                              [[2, O], [1, 2], [2 * O, NCORES]])
                nc.sync.dma_start(sg8[:], src)
                sg = ppool.tile([O, 2], F32, tag="sg")
                nc.vector.tensor_reduce(sg[:], sg8[:], AX.X, AL.add)
                denom = float(NPIX_TOT)
            else:
                sg = s1
                denom = float(T * W)

            mean = ppool.tile([O, 1], F32, tag="mean")
            TS(mean[:], sg[:, 0:1], 1.0 / denom, None, AL.mult)
            ex2 = ppool.tile([O, 1], F32, tag="ex2")
            TS(ex2[:], sg[:, 1:2], 1.0 / denom, None, AL.mult)
            m2 = ppool.tile([O, 1], F32, tag="m2")
            TT(m2[:], mean[:], mean[:], AL.mult)
            var = ppool.tile([O, 1], F32, tag="var")
            TT(var[:], ex2[:], m2[:], AL.subtract)
            stdt = ppool.tile([O, 1], F32, tag="stdt")
            nc.scalar.activation(stdt[:], var[:], AF.Sqrt, bias=epst[0:O, :])
            rstd = ppool.tile([O, 1], F32, tag="rstd")
            nc.vector.reciprocal(rstd[:], stdt[:])
            scl = ppool.tile([O, 1], F32, tag="scl")
            TT(scl[:], gbs[:, 0:1], rstd[:], AL.mult)
            msc = ppool.tile([O, 1], F32, tag="msc")
            TT(msc[:], mean[:], scl[:], AL.mult)
            sh = ppool.tile([O, 1], F32, tag="sh")
            TT(sh[:], gbs[:, 1:2], msc[:], AL.subtract)

            # BN apply + ReLU in 4 chunks, overlapped with output DMA
            CH = T // 4
            for cidx in range(4):
                tlo = cidx * CH
                nc.scalar.activation(ON[:, tlo : tlo + CH, :],
                                     OPRE[:, tlo : tlo + CH, :],
                                     AF.Relu, bias=sh[:], scale=scl[:])
                nc.sync.dma_start(outd[:, tlo : tlo + CH, :],
                                  ON[:, tlo : tlo + CH, :])

    if fixup:
        fix_multiwait(nc)
    return nc


# ---------------- host-side preparation ----------------

def _host_prep(x, conv_w, off_w, off_b, mask_w, mask_b, gamma, beta, T=RPC):
    """Build the 8 per-core input maps."""
    x = np.asarray(x, np.float32)

    # padded parity patch buffer, bf16, rows [p][c][q]
    xcl = np.transpose(x, (0, 2, 3, 1)).astype(BF)      # [B, H, W, C]
    xpad = np.zeros((B, 138, 138, C), BF)
    xpad[:, 4 : 4 + H, 4 : 4 + W] = xcl
    PB = np.zeros((B, 4, GRID, GRID, 2, C, 2), BF)
    for P in range(2):
        for Q in range(2):
            for p in range(2):
                for q in range(2):
                    sub = xpad[:, P + p : P + p + 2 * GRID : 2,
                               Q + q : Q + q + 2 * GRID : 2, :]
                    PB[:, 2 * P + Q, :, :, p, :, q] = sub
    PB = PB.reshape(B, PBROWS, 256)

    # wcat: [C+1, 9, 27]; channel rows = cat(off_w, mask_w) transposed;
    # ones row = biases at k=0
    wfull = np.concatenate([off_w, mask_w], axis=0)     # [27, C, 3, 3]
    wcat = np.zeros((C + 1, K9, 27), BF)
    wcat[:C] = np.transpose(wfull.reshape(27, C, K9), (1, 2, 0)).astype(BF)
    bias = np.concatenate([off_b, mask_b]).astype(BF)   # [27]
    wcat[C, 0, :] = bias

    # w2q: [128, 9, O] with partition j = c*2 + q -> w2[c, k, o] duplicated
    w2 = np.transpose(np.asarray(conv_w, np.float32).reshape(O, C, K9),
                      (1, 2, 0))                        # [C, 9, O]
    w2kco = np.transpose(w2, (1, 0, 2)).astype(BF)      # [9, C, O]
    w2q = np.zeros((128, K9, O), BF)
    w2q[0::2] = np.transpose(w2kco, (1, 0, 2))
    w2q[1::2] = np.transpose(w2kco, (1, 0, 2))

    ident = np.eye(128, dtype=np.float32).astype(BF)
    gb = np.stack([np.asarray(gamma, np.float32),
                   np.asarray(beta, np.float32)], axis=1)

    ky = np.repeat(np.arange(3), 3).astype(np.float32)
    kx = np.tile(np.arange(3), 3).astype(np.float32)
    gx = np.arange(128, dtype=np.float32)

    in_maps = []
    for core in range(NCORES):
        b, strip = divmod(core, 4)
        r0 = strip * RPC
        # xslab [C+1, T+2, W+2]: rows r0-1 .. r0+T, zero padded, ones row
        xslab = np.zeros((C + 1, T + 2, W + 2), BF)
        lo, hi = r0 - 1, r0 + T + 1
        glo, ghi = max(lo, 0), min(hi, H)
        xslab[:C, (glo - lo) : (ghi - lo), 1 : W + 1] = x[b, :, glo:ghi, :].astype(BF)
        xslab[C] = 1.0
        cyh = (r0 + np.arange(T)[None, :, None] + (ky - 1.0)[None, None, :]
               + np.zeros((128, 1, 1))).astype(np.float32)
        cxh = (gx[:, None, None] + (kx - 1.0)[None, None, :]
               + np.zeros((1, T, 1))).astype(np.float32)
        in_maps.append({
            "xslab": xslab, "pbc": PB[b],
            "cy": cyh, "cx": cxh, "wcat": wcat, "w2q": w2q,
            "ident": ident, "gb": gb,
        })
    return in_maps


_NC_CACHE = {}


def kernel(x, conv_w, off_w, off_b, mask_w, mask_b, gamma, beta):
    if "nc" not in _NC_CACHE:
        _NC_CACHE["nc"] = build_nc()
    nc = _NC_CACHE["nc"]
    in_maps = _host_prep(x, conv_w, off_w, off_b, mask_w, mask_b, gamma, beta)
    res = run_bass_kernel_spmd(nc, in_maps, core_ids=list(range(NCORES)))
    out = np.zeros((B, O, H, W), np.float32)
    for core in range(NCORES):
        b, strip = divmod(core, 4)
        r0 = strip * RPC
        out[b, :, r0 : r0 + RPC, :] = res.results[core]["outn"]
    return out
